# revision 6
# baseline (speedup 1.0000x reference)
"""SpecAugment (log-mel masking) Trainium2 kernel.

Full inputs: x [64,128,3000] f32, f0/f_w/t0/t_w [64,2] i32.
out[b,f,t] = fill_b if (f in freq band) or (t in time band) else x[b,f,t],
fill_b = min over x[b].

Strategy: batch-shard B=64 across 8 cores (8 samples/core). The int mask
params are tiny host tensors, so the per-sample mask vectors are computed
on host (numpy) and shipped as data; the device does only the
memory-bound work:
  per sample: DMA x[b] -> SBUF; DVE reduce_min (free axis) -> [128,1];
  tiny DMA gather -> [1,128]; reduce_min -> fill [1,1]; DMA bcast ->
  [128,1]; ScalarE blends the freq(row) mask via per-partition
  scale/bias; PE broadcasts fill*time_mask into PSUM via K=1 matmuls;
  DVE copy_predicated overwrites time-masked cols with fill; DMA out.
HBM traffic is the minimum 2 x 12.3MB per core -> ~69us roofline.
"""

import numpy as np

import concourse.bacc as bacc
import concourse.bass as bass
import concourse.mybir as mybir
import concourse.tile as tile
import concourse.bass_utils as bass_utils

B, F, T = 64, 128, 3000
N_CORES = 8
BPC = B // N_CORES  # samples per core
F32 = mybir.dt.float32

_cached = {}


def _build_nc():
    nc = bacc.Bacc("TRN2", target_bir_lowering=False, debug=False)
    x = nc.dram_tensor("x_sh", [BPC, F, T], F32, kind="ExternalInput")
    mt = nc.dram_tensor("mt_sh", [BPC, T], F32, kind="ExternalInput")
    kf2 = nc.dram_tensor("kf2_sh", [BPC, F, 2], F32, kind="ExternalInput")
    y = nc.dram_tensor("y_sh", [BPC, F, T], F32, kind="ExternalOutput")

    xa, ma, ka, ya = x.ap(), mt.ap(), kf2.ap(), y.ap()

    with tile.TileContext(nc) as tc:
        with (
            tc.tile_pool(name="xp", bufs=3) as xp,
            tc.tile_pool(name="row", bufs=4) as rowp,
            tc.tile_pool(name="small", bufs=4) as sp,
            tc.tile_pool(name="single", bufs=1) as single,
            tc.tile_pool(name="ps", bufs=1, space="PSUM") as psp,
            tc.tile_pool(name="ps_small", bufs=2, space="PSUM") as psps,
        ):
            ones_row = single.tile([1, F], F32)
            nc.vector.memset(ones_row, 1.0)
            one11 = single.tile([1, 1], F32)
            nc.vector.memset(one11, 1.0)

            for b in range(BPC):
                xt = xp.tile([F, T], F32, tag="xt")
                nc.sync.dma_start(out=xt, in_=xa[b])
                mtb = rowp.tile([1, T], F32, tag="mtb")
                nc.gpsimd.dma_start(out=mtb, in_=ma[b : b + 1])
                kfb = sp.tile([F, 2], F32, tag="kfb")
                nc.gpsimd.dma_start(out=kfb, in_=ka[b])

                # per-sample min: free-axis reduce, gather across partitions
                colmin = sp.tile([F, 1], F32, tag="colmin")
                nc.vector.tensor_reduce(
                    out=colmin, in_=xt, axis=mybir.AxisListType.X,
                    op=mybir.AluOpType.min,
                )
                rowmin = sp.tile([1, F], F32, tag="rowmin")
                nc.gpsimd.dma_start(out=rowmin, in_=colmin)
                fill11 = sp.tile([1, 1], F32, tag="fill11")
                nc.vector.tensor_reduce(
                    out=fill11, in_=rowmin, axis=mybir.AxisListType.X,
                    op=mybir.AluOpType.min,
                )
                # fill broadcast [1,1] -> [1,128] (free) -> [128,1] (PE)
                fill_row = sp.tile([1, F], F32, tag="fill_row")
                nc.vector.tensor_scalar_mul(
                    out=fill_row, in0=ones_row, scalar1=fill11,
                )
                fill128_ps = psps.tile([F, 1], F32, tag="fill128_ps")
                nc.tensor.matmul(fill128_ps, fill_row, one11, start=True, stop=True)
                fill128 = sp.tile([F, 1], F32, tag="fill128")
                nc.vector.tensor_copy(fill128, fill128_ps)

                # freq(row) mask on ScalarE: x = x*keep_f + fill*(1-keep_f)
                fillc = sp.tile([F, 1], F32, tag="fillc")
                nc.vector.tensor_tensor(
                    out=fillc, in0=fill128, in1=kfb[:, 1:2],
                    op=mybir.AluOpType.mult,
                )
                nc.scalar.activation(
                    out=xt, in_=xt,
                    func=mybir.ActivationFunctionType.Identity,
                    bias=fillc, scale=kfb[:, 0:1],
                )

                # time(col) mask: PE broadcasts fill*mt into PSUM,
                # predicated copy overwrites masked cols with fill
                ms = psp.tile([F, T], F32, tag="ms")
                for c0 in range(0, T, 512):
                    cw = min(512, T - c0)
                    nc.tensor.matmul(
                        ms[:, c0 : c0 + cw],
                        fill_row,
                        mtb[:, c0 : c0 + cw],
                        start=True,
                        stop=True,
                    )
                nc.vector.copy_predicated(
                    out=xt,
                    mask=ms.bitcast(mybir.dt.int32),
                    data=fill128.to_broadcast([F, T]),
                )

                nc.scalar.dma_start(out=ya[b], in_=xt)
    nc.compile()
    return nc


def _host_masks(f0, f_w, t0, t_w):
    fidx = np.arange(F, dtype=np.int32)
    tidx = np.arange(T, dtype=np.int32)
    fm = (
        (fidx[None, None, :] >= f0[:, :, None])
        & (fidx[None, None, :] < (f0 + f_w)[:, :, None])
    ).any(axis=1)  # [B,F] bool
    tm = (
        (tidx[None, None, :] >= t0[:, :, None])
        & (tidx[None, None, :] < (t0 + t_w)[:, :, None])
    ).any(axis=1)  # [B,T] bool
    mt = tm.astype(np.float32)
    kf2 = np.stack(
        [(~fm).astype(np.float32), fm.astype(np.float32)], axis=-1
    )  # [B,F,2]
    return mt, kf2


def kernel(x, f0, f_w, t0, t_w, **_):
    x = np.ascontiguousarray(np.asarray(x, dtype=np.float32))
    f0 = np.asarray(f0)
    f_w = np.asarray(f_w)
    t0 = np.asarray(t0)
    t_w = np.asarray(t_w)
    mt, kf2 = _host_masks(f0, f_w, t0, t_w)

    if "nc" not in _cached:
        _cached["nc"] = _build_nc()
    nc = _cached["nc"]

    in_maps = []
    for c in range(N_CORES):
        s = slice(c * BPC, (c + 1) * BPC)
        in_maps.append(
            {
                "x_sh": np.ascontiguousarray(x[s]),
                "mt_sh": np.ascontiguousarray(mt[s]),
                "kf2_sh": np.ascontiguousarray(kf2[s]),
            }
        )
    res = bass_utils.run_bass_kernel_spmd(
        nc, in_maps, core_ids=list(range(N_CORES))
    )
    out = np.concatenate([r["y_sh"] for r in res.results], axis=0)
    return out


# revision 9
# speedup vs baseline: 1.3516x; 1.3516x over previous
"""SpecAugment (log-mel masking) Trainium2 kernel.

Full inputs: x [64,128,3000] f32, f0/f_w/t0/t_w [64,2] i32.
out[b,f,t] = fill_b if (f in freq band) or (t in time band) else x[b,f,t],
fill_b = min over x[b].

Strategy: batch-shard B=64 across 8 cores (8 samples/core). The int mask
params are tiny host tensors, so the per-sample mask vectors are computed
on host (numpy) and shipped as data; the device does only the
memory-bound work:
  per sample: DMA x[b] -> SBUF; DVE reduce_min (free axis) -> [128,1];
  tiny DMA gather -> [1,128]; reduce_min -> fill [1,1]; DMA bcast ->
  [128,1]; ScalarE blends the freq(row) mask via per-partition
  scale/bias; PE broadcasts fill*time_mask into PSUM via K=1 matmuls;
  DVE copy_predicated overwrites time-masked cols with fill; DMA out.
HBM traffic is the minimum 2 x 12.3MB per core -> ~69us roofline.
"""

import ml_dtypes
import numpy as np

import concourse.bacc as bacc
import concourse.bass as bass
import concourse.mybir as mybir
import concourse.tile as tile
import concourse.bass_utils as bass_utils

B, F, T = 64, 128, 3000
N_CORES = 8
BPC = B // N_CORES  # samples per core
F32 = mybir.dt.float32

_cached = {}


def _build_nc():
    nc = bacc.Bacc("TRN2", target_bir_lowering=False, debug=False)
    x = nc.dram_tensor("x_sh", [BPC, F, T], F32, kind="ExternalInput")
    mt = nc.dram_tensor("mt_sh", [BPC, T], mybir.dt.bfloat16, kind="ExternalInput")
    kf2 = nc.dram_tensor("kf2_sh", [BPC, F, 2], F32, kind="ExternalInput")
    y = nc.dram_tensor("y_sh", [BPC, F, T], F32, kind="ExternalOutput")

    xa, ma, ka, ya = x.ap(), mt.ap(), kf2.ap(), y.ap()

    with tile.TileContext(nc) as tc:
        with (
            tc.tile_pool(name="xp", bufs=3) as xp,
            tc.tile_pool(name="row", bufs=4) as rowp,
            tc.tile_pool(name="small", bufs=4) as sp,
            tc.tile_pool(name="single", bufs=1) as single,
            tc.tile_pool(name="ps", bufs=1, space="PSUM") as psp,
            tc.tile_pool(name="ps_small", bufs=2, space="PSUM") as psps,
        ):
            ones_row = single.tile([1, F], F32)
            nc.vector.memset(ones_row, 1.0)
            ones_row_bf = single.tile([1, F], mybir.dt.bfloat16)
            nc.vector.memset(ones_row_bf, 1.0)
            one11 = single.tile([1, 1], F32)
            nc.vector.memset(one11, 1.0)

            for b in range(BPC):
                xt = xp.tile([F, T], F32, tag="xt")
                nc.sync.dma_start(out=xt, in_=xa[b])
                mtb = rowp.tile([1, T], mybir.dt.bfloat16, tag="mtb")
                nc.gpsimd.dma_start(out=mtb, in_=ma[b : b + 1])
                kfb = sp.tile([F, 2], F32, tag="kfb")
                nc.gpsimd.dma_start(out=kfb, in_=ka[b])

                # per-sample min: free-axis reduce, gather across partitions
                colmin = sp.tile([F, 1], F32, tag="colmin")
                nc.vector.tensor_reduce(
                    out=colmin, in_=xt, axis=mybir.AxisListType.X,
                    op=mybir.AluOpType.min,
                )
                rowmin = sp.tile([1, F], F32, tag="rowmin")
                nc.gpsimd.dma_start(out=rowmin, in_=colmin)
                fill11 = sp.tile([1, 1], F32, tag="fill11")
                nc.vector.tensor_reduce(
                    out=fill11, in_=rowmin, axis=mybir.AxisListType.X,
                    op=mybir.AluOpType.min,
                )
                # fill broadcast [1,1] -> [1,128] (free) -> [128,1] (PE)
                fill_row = sp.tile([1, F], F32, tag="fill_row")
                nc.vector.tensor_scalar_mul(
                    out=fill_row, in0=ones_row, scalar1=fill11,
                )
                fill128_ps = psps.tile([F, 1], F32, tag="fill128_ps")
                nc.tensor.matmul(fill128_ps, fill_row, one11, start=True, stop=True)
                fill128 = sp.tile([F, 1], F32, tag="fill128")
                nc.vector.tensor_copy(fill128, fill128_ps)

                # freq(row) mask on ScalarE: x = x*keep_f + fill*(1-keep_f)
                fillc = sp.tile([F, 1], F32, tag="fillc")
                nc.vector.tensor_tensor(
                    out=fillc, in0=fill128, in1=kfb[:, 1:2],
                    op=mybir.AluOpType.mult,
                )
                nc.scalar.activation(
                    out=xt, in_=xt,
                    func=mybir.ActivationFunctionType.Identity,
                    bias=fillc, scale=kfb[:, 0:1],
                )

                # time(col) mask: PE broadcasts fill*mt into PSUM,
                # predicated copy overwrites masked cols with fill
                ms = psp.tile([F, T], F32, tag="ms")
                for c0 in range(0, T, 512):
                    cw = min(512, T - c0)
                    nc.tensor.matmul(
                        ms[:, c0 : c0 + cw],
                        ones_row_bf,
                        mtb[:, c0 : c0 + cw],
                        start=True,
                        stop=True,
                    )
                nc.vector.copy_predicated(
                    out=xt,
                    mask=ms.bitcast(mybir.dt.int32),
                    data=fill128.to_broadcast([F, T]),
                )

                nc.scalar.dma_start(out=ya[b], in_=xt)
    nc.compile()
    return nc


def _host_masks(f0, f_w, t0, t_w):
    fidx = np.arange(F, dtype=np.int32)
    tidx = np.arange(T, dtype=np.int32)
    fm = (
        (fidx[None, None, :] >= f0[:, :, None])
        & (fidx[None, None, :] < (f0 + f_w)[:, :, None])
    ).any(axis=1)  # [B,F] bool
    tm = (
        (tidx[None, None, :] >= t0[:, :, None])
        & (tidx[None, None, :] < (t0 + t_w)[:, :, None])
    ).any(axis=1)  # [B,T] bool
    mt = tm.astype(np.float32)
    kf2 = np.stack(
        [(~fm).astype(np.float32), fm.astype(np.float32)], axis=-1
    )  # [B,F,2]
    return mt, kf2


def kernel(x, f0, f_w, t0, t_w, **_):
    x = np.ascontiguousarray(np.asarray(x, dtype=np.float32))
    f0 = np.asarray(f0)
    f_w = np.asarray(f_w)
    t0 = np.asarray(t0)
    t_w = np.asarray(t_w)
    mt, kf2 = _host_masks(f0, f_w, t0, t_w)

    if "nc" not in _cached:
        _cached["nc"] = _build_nc()
    nc = _cached["nc"]

    in_maps = []
    for c in range(N_CORES):
        s = slice(c * BPC, (c + 1) * BPC)
        in_maps.append(
            {
                "x_sh": np.ascontiguousarray(x[s]),
                "mt_sh": np.ascontiguousarray(mt[s]).astype(ml_dtypes.bfloat16),
                "kf2_sh": np.ascontiguousarray(kf2[s]),
            }
        )
    res = bass_utils.run_bass_kernel_spmd(
        nc, in_maps, core_ids=list(range(N_CORES))
    )
    out = np.concatenate([r["y_sh"] for r in res.results], axis=0)
    return out


# revision 10
# speedup vs baseline: 1.4159x; 1.0476x over previous
"""SpecAugment (log-mel masking) Trainium2 kernel.

Full inputs: x [64,128,3000] f32, f0/f_w/t0/t_w [64,2] i32.
out[b,f,t] = fill_b if (f in freq band) or (t in time band) else x[b,f,t],
fill_b = min over x[b].

Strategy: batch-shard B=64 across 8 cores (8 samples/core). The int mask
params are tiny host tensors, so the per-sample mask vectors are computed
on host (numpy) and shipped as data; the device does only the
memory-bound work:
  per sample: DMA x[b] -> SBUF; DVE reduce_min (free axis) -> [128,1];
  tiny DMA gather -> [1,128]; reduce_min -> fill [1,1]; DMA bcast ->
  [128,1]; ScalarE blends the freq(row) mask via per-partition
  scale/bias; PE broadcasts fill*time_mask into PSUM via K=1 matmuls;
  DVE copy_predicated overwrites time-masked cols with fill; DMA out.
HBM traffic is the minimum 2 x 12.3MB per core -> ~69us roofline.
"""

import ml_dtypes
import numpy as np

import concourse.bacc as bacc
import concourse.bass as bass
import concourse.mybir as mybir
import concourse.tile as tile
import concourse.bass_utils as bass_utils

B, F, T = 64, 128, 3000
N_CORES = 8
BPC = B // N_CORES  # samples per core
F32 = mybir.dt.float32

_cached = {}


def _build_nc():
    nc = bacc.Bacc("TRN2", target_bir_lowering=False, debug=False)
    x = nc.dram_tensor("x_sh", [BPC, F, T], F32, kind="ExternalInput")
    mt = nc.dram_tensor("mt_sh", [BPC, T], mybir.dt.bfloat16, kind="ExternalInput")
    kf2 = nc.dram_tensor("kf2_sh", [BPC, F, 2], F32, kind="ExternalInput")
    y = nc.dram_tensor("y_sh", [BPC, F, T], F32, kind="ExternalOutput")

    xa, ma, ka, ya = x.ap(), mt.ap(), kf2.ap(), y.ap()

    with tile.TileContext(nc) as tc:
        with (
            tc.tile_pool(name="xp", bufs=5) as xp,
            tc.tile_pool(name="row", bufs=6) as rowp,
            tc.tile_pool(name="small", bufs=6) as sp,
            tc.tile_pool(name="single", bufs=1) as single,
            tc.tile_pool(name="ps", bufs=2, space="PSUM") as psp,
            tc.tile_pool(name="ps_small", bufs=2, space="PSUM") as psps,
        ):
            ones_row = single.tile([1, F], F32)
            nc.vector.memset(ones_row, 1.0)
            ones_row_bf = single.tile([1, F], mybir.dt.bfloat16)
            nc.vector.memset(ones_row_bf, 1.0)
            one11 = single.tile([1, 1], F32)
            nc.vector.memset(one11, 1.0)

            for b in range(BPC):
                xt = xp.tile([F, T], F32, tag="xt")
                nc.sync.dma_start(out=xt, in_=xa[b])
                mtb = rowp.tile([1, T], mybir.dt.bfloat16, tag="mtb")
                nc.gpsimd.dma_start(out=mtb, in_=ma[b : b + 1])
                kfb = sp.tile([F, 2], F32, tag="kfb")
                nc.gpsimd.dma_start(out=kfb, in_=ka[b])

                # per-sample min: free-axis reduce, gather across partitions
                colmin = sp.tile([F, 1], F32, tag="colmin")
                nc.vector.tensor_reduce(
                    out=colmin, in_=xt, axis=mybir.AxisListType.X,
                    op=mybir.AluOpType.min,
                )
                rowmin = sp.tile([1, F], F32, tag="rowmin")
                nc.gpsimd.dma_start(out=rowmin, in_=colmin)
                fill11 = sp.tile([1, 1], F32, tag="fill11")
                nc.vector.tensor_reduce(
                    out=fill11, in_=rowmin, axis=mybir.AxisListType.X,
                    op=mybir.AluOpType.min,
                )
                # fill broadcast [1,1] -> [1,128] (free) -> [128,1] (PE)
                fill_row = sp.tile([1, F], F32, tag="fill_row")
                nc.scalar.mul(fill_row, ones_row, fill11)
                fill128_ps = psps.tile([F, 1], F32, tag="fill128_ps")
                nc.tensor.matmul(fill128_ps, fill_row, one11, start=True, stop=True)
                fill128 = sp.tile([F, 1], F32, tag="fill128")
                nc.scalar.copy(fill128, fill128_ps)

                # freq(row) mask on ScalarE: x = x*keep_f + fill*(1-keep_f)
                fillc = sp.tile([F, 1], F32, tag="fillc")
                nc.gpsimd.tensor_tensor(
                    out=fillc, in0=fill128, in1=kfb[:, 1:2],
                    op=mybir.AluOpType.mult,
                )
                nc.scalar.activation(
                    out=xt, in_=xt,
                    func=mybir.ActivationFunctionType.Identity,
                    bias=fillc, scale=kfb[:, 0:1],
                )

                # time(col) mask: PE broadcasts fill*mt into PSUM,
                # predicated copy overwrites masked cols with fill
                H = T // 2
                for h in range(2):
                    msh = psp.tile([F, H], F32, tag="ms")
                    for c0 in range(0, H, 512):
                        cw = min(512, H - c0)
                        nc.tensor.matmul(
                            msh[:, c0 : c0 + cw],
                            ones_row_bf,
                            mtb[:, h * H + c0 : h * H + c0 + cw],
                            start=True,
                            stop=True,
                        )
                    nc.vector.copy_predicated(
                        out=xt[:, h * H : (h + 1) * H],
                        mask=msh.bitcast(mybir.dt.int32),
                        data=fill128.to_broadcast([F, H]),
                    )

                nc.scalar.dma_start(out=ya[b], in_=xt)
    nc.compile()
    return nc


def _host_masks(f0, f_w, t0, t_w):
    fidx = np.arange(F, dtype=np.int32)
    tidx = np.arange(T, dtype=np.int32)
    fm = (
        (fidx[None, None, :] >= f0[:, :, None])
        & (fidx[None, None, :] < (f0 + f_w)[:, :, None])
    ).any(axis=1)  # [B,F] bool
    tm = (
        (tidx[None, None, :] >= t0[:, :, None])
        & (tidx[None, None, :] < (t0 + t_w)[:, :, None])
    ).any(axis=1)  # [B,T] bool
    mt = tm.astype(np.float32)
    kf2 = np.stack(
        [(~fm).astype(np.float32), fm.astype(np.float32)], axis=-1
    )  # [B,F,2]
    return mt, kf2


def kernel(x, f0, f_w, t0, t_w, **_):
    x = np.ascontiguousarray(np.asarray(x, dtype=np.float32))
    f0 = np.asarray(f0)
    f_w = np.asarray(f_w)
    t0 = np.asarray(t0)
    t_w = np.asarray(t_w)
    mt, kf2 = _host_masks(f0, f_w, t0, t_w)

    if "nc" not in _cached:
        _cached["nc"] = _build_nc()
    nc = _cached["nc"]

    in_maps = []
    for c in range(N_CORES):
        s = slice(c * BPC, (c + 1) * BPC)
        in_maps.append(
            {
                "x_sh": np.ascontiguousarray(x[s]),
                "mt_sh": np.ascontiguousarray(mt[s]).astype(ml_dtypes.bfloat16),
                "kf2_sh": np.ascontiguousarray(kf2[s]),
            }
        )
    res = bass_utils.run_bass_kernel_spmd(
        nc, in_maps, core_ids=list(range(N_CORES))
    )
    out = np.concatenate([r["y_sh"] for r in res.results], axis=0)
    return out


# revision 11
# speedup vs baseline: 1.5221x; 1.0750x over previous
"""SpecAugment (log-mel masking) Trainium2 kernel.

Full inputs: x [64,128,3000] f32, f0/f_w/t0/t_w [64,2] i32.
out[b,f,t] = fill_b if (f in freq band) or (t in time band) else x[b,f,t],
fill_b = min over x[b].

Strategy: batch-shard B=64 across 8 cores (8 samples/core). The int mask
params are tiny host tensors, so the per-sample 0/1 mask vectors are
computed on host and shipped as bf16 data; the device does only the
memory-bound work. Per sample:
  - DMA x[b] [128,3000] -> SBUF
  - DVE reduce_min (free axis) -> [128,1]; tiny DMA gather -> [1,128];
    reduce_min -> fill [1,1]; broadcast to [128,1] via tiny PE matmul
  - combined mask = ones(x)mt + mf(x)ones as ONE K=2 bf16 matmul per
    512-col chunk into PSUM (values {0,1,2}; nonzero == masked)
  - DVE copy_predicated overwrites masked cells with fill (data operand
    is fill128 broadcast along the free axis)
  - DMA xt -> y[b]
HBM traffic is the minimum 2 x 12.3MB per core -> ~69us roofline.
"""

import ml_dtypes
import numpy as np

import concourse.bacc as bacc
import concourse.bass as bass
import concourse.mybir as mybir
import concourse.tile as tile
import concourse.bass_utils as bass_utils

B, F, T = 64, 128, 3000
N_CORES = 8
BPC = B // N_CORES  # samples per core
F32 = mybir.dt.float32
BF16 = mybir.dt.bfloat16

_cached = {}


def _build_nc():
    nc = bacc.Bacc("TRN2", target_bir_lowering=False, debug=False)
    x = nc.dram_tensor("x_sh", [BPC, F, T], F32, kind="ExternalInput")
    # row0 = time mask (0/1), row1 = ones
    mtr = nc.dram_tensor("mtr_sh", [BPC, 2, T], BF16, kind="ExternalInput")
    # row0 = ones, row1 = freq mask (0/1)
    mfl = nc.dram_tensor("mfl_sh", [BPC, 2, F], BF16, kind="ExternalInput")
    y = nc.dram_tensor("y_sh", [BPC, F, T], F32, kind="ExternalOutput")

    xa, ta, fa, ya = x.ap(), mtr.ap(), mfl.ap(), y.ap()

    H = T // 2

    with tile.TileContext(nc) as tc:
        with (
            tc.tile_pool(name="xp", bufs=5) as xp,
            tc.tile_pool(name="row", bufs=6) as rowp,
            tc.tile_pool(name="small", bufs=6) as sp,
            tc.tile_pool(name="single", bufs=1) as single,
            tc.tile_pool(name="ps", bufs=2, space="PSUM") as psp,
            tc.tile_pool(name="ps_small", bufs=2, space="PSUM") as psps,
        ):
            ones_row = single.tile([1, F], F32)
            nc.vector.memset(ones_row, 1.0)
            one11 = single.tile([1, 1], F32)
            nc.vector.memset(one11, 1.0)

            for b in range(BPC):
                xt = xp.tile([F, T], F32, tag="xt")
                nc.sync.dma_start(out=xt, in_=xa[b])
                mtb = rowp.tile([2, T], BF16, tag="mtb")
                nc.gpsimd.dma_start(out=mtb, in_=ta[b])
                mfb = sp.tile([2, F], BF16, tag="mfb")
                nc.gpsimd.dma_start(out=mfb, in_=fa[b])

                # per-sample min: free-axis reduce, gather across partitions
                colmin = sp.tile([F, 1], F32, tag="colmin")
                nc.vector.tensor_reduce(
                    out=colmin, in_=xt, axis=mybir.AxisListType.X,
                    op=mybir.AluOpType.min,
                )
                rowmin = sp.tile([1, F], F32, tag="rowmin")
                nc.gpsimd.dma_start(out=rowmin, in_=colmin)
                fill11 = sp.tile([1, 1], F32, tag="fill11")
                nc.vector.tensor_reduce(
                    out=fill11, in_=rowmin, axis=mybir.AxisListType.X,
                    op=mybir.AluOpType.min,
                )
                # fill broadcast [1,1] -> [1,128] (free) -> [128,1] (PE)
                fill_row = sp.tile([1, F], F32, tag="fill_row")
                nc.scalar.mul(fill_row, ones_row, fill11)
                fill128_ps = psps.tile([F, 1], F32, tag="fill128_ps")
                nc.tensor.matmul(fill128_ps, fill_row, one11, start=True, stop=True)
                fill128 = sp.tile([F, 1], F32, tag="fill128")
                nc.scalar.copy(fill128, fill128_ps)

                # combined mask: ones(x)mt + mf(x)ones via K=2 bf16 matmuls;
                # nonzero => masked. Two half-T PSUM tiles for PE/DVE overlap.
                for h in range(2):
                    msh = psp.tile([F, H], F32, tag="ms")
                    for c0 in range(0, H, 512):
                        cw = min(512, H - c0)
                        nc.tensor.matmul(
                            msh[:, c0 : c0 + cw],
                            mfb,
                            mtb[:, h * H + c0 : h * H + c0 + cw],
                            start=True,
                            stop=True,
                        )
                    nc.vector.copy_predicated(
                        out=xt[:, h * H : (h + 1) * H],
                        mask=msh.bitcast(mybir.dt.int32),
                        data=fill128.to_broadcast([F, H]),
                    )

                nc.scalar.dma_start(out=ya[b], in_=xt)
    nc.compile()
    return nc


def _host_masks(f0, f_w, t0, t_w):
    nb = f0.shape[0]
    fidx = np.arange(F, dtype=np.int32)
    tidx = np.arange(T, dtype=np.int32)
    fm = (
        (fidx[None, None, :] >= f0[:, :, None])
        & (fidx[None, None, :] < (f0 + f_w)[:, :, None])
    ).any(axis=1)  # [B,F] bool
    tm = (
        (tidx[None, None, :] >= t0[:, :, None])
        & (tidx[None, None, :] < (t0 + t_w)[:, :, None])
    ).any(axis=1)  # [B,T] bool
    mtr = np.ones((nb, 2, T), np.float32)
    mtr[:, 0, :] = tm
    mfl = np.ones((nb, 2, F), np.float32)
    mfl[:, 1, :] = fm
    return mtr.astype(ml_dtypes.bfloat16), mfl.astype(ml_dtypes.bfloat16)


def kernel(x, f0, f_w, t0, t_w, **_):
    x = np.ascontiguousarray(np.asarray(x, dtype=np.float32))
    f0 = np.asarray(f0)
    f_w = np.asarray(f_w)
    t0 = np.asarray(t0)
    t_w = np.asarray(t_w)
    mtr, mfl = _host_masks(f0, f_w, t0, t_w)

    if "nc" not in _cached:
        _cached["nc"] = _build_nc()
    nc = _cached["nc"]

    in_maps = []
    for c in range(N_CORES):
        s = slice(c * BPC, (c + 1) * BPC)
        in_maps.append(
            {
                "x_sh": np.ascontiguousarray(x[s]),
                "mtr_sh": np.ascontiguousarray(mtr[s]),
                "mfl_sh": np.ascontiguousarray(mfl[s]),
            }
        )
    res = bass_utils.run_bass_kernel_spmd(
        nc, in_maps, core_ids=list(range(N_CORES))
    )
    out = np.concatenate([r["y_sh"] for r in res.results], axis=0)
    return out


# revision 12
# speedup vs baseline: 1.5555x; 1.0219x over previous
"""SpecAugment (log-mel masking) Trainium2 kernel.

Full inputs: x [64,128,3000] f32, f0/f_w/t0/t_w [64,2] i32.
out[b,f,t] = fill_b if (f in freq band) or (t in time band) else x[b,f,t],
fill_b = min over x[b].

Strategy: batch-shard B=64 across 8 cores (8 samples/core). The int mask
params are tiny host tensors, so the per-sample 0/1 mask vectors are
computed on host and shipped as bf16 data; the device does only the
memory-bound work. Per sample:
  - DMA x[b] [128,3000] -> SBUF
  - DVE reduce_min (free axis) -> [128,1]; tiny DMA gather -> [1,128];
    reduce_min -> fill [1,1]; broadcast to [128,1] via tiny PE matmul
  - combined mask = ones(x)mt + mf(x)ones as ONE K=2 bf16 matmul per
    512-col chunk into PSUM (values {0,1,2}; nonzero == masked)
  - DVE copy_predicated overwrites masked cells with fill (data operand
    is fill128 broadcast along the free axis)
  - DMA xt -> y[b]
HBM traffic is the minimum 2 x 12.3MB per core -> ~69us roofline.
"""

import ml_dtypes
import numpy as np

import concourse.bacc as bacc
import concourse.bass as bass
import concourse.mybir as mybir
import concourse.tile as tile
import concourse.bass_utils as bass_utils

B, F, T = 64, 128, 3000
N_CORES = 8
BPC = B // N_CORES  # samples per core
F32 = mybir.dt.float32
BF16 = mybir.dt.bfloat16

_cached = {}


def _build_nc():
    nc = bacc.Bacc("TRN2", target_bir_lowering=False, debug=False)
    x = nc.dram_tensor("x_sh", [BPC, F, T], F32, kind="ExternalInput")
    # row0 = time mask (0/1), row1 = ones
    mtr = nc.dram_tensor("mtr_sh", [BPC, 2, T], BF16, kind="ExternalInput")
    # row0 = ones, row1 = freq mask (0/1)
    mfl = nc.dram_tensor("mfl_sh", [BPC, 2, F], BF16, kind="ExternalInput")
    y = nc.dram_tensor("y_sh", [BPC, F, T], F32, kind="ExternalOutput")

    xa, ta, fa, ya = x.ap(), mtr.ap(), mfl.ap(), y.ap()

    H = T // 2

    with tile.TileContext(nc) as tc:
        with (
            tc.tile_pool(name="xp", bufs=6) as xp,
            tc.tile_pool(name="row", bufs=6) as rowp,
            tc.tile_pool(name="small", bufs=6) as sp,
            tc.tile_pool(name="single", bufs=1) as single,
            tc.tile_pool(name="ps", bufs=2, space="PSUM") as psp,
            tc.tile_pool(name="ps_small", bufs=2, space="PSUM") as psps,
        ):
            ones_row = single.tile([1, F], F32)
            nc.vector.memset(ones_row, 1.0)
            one11 = single.tile([1, 1], F32)
            nc.vector.memset(one11, 1.0)

            for b in range(BPC):
                xt = xp.tile([F, T], F32, tag="xt")
                nc.sync.dma_start(out=xt, in_=xa[b])
                mtb = rowp.tile([2, T], BF16, tag="mtb")
                nc.gpsimd.dma_start(out=mtb, in_=ta[b])
                mfb = sp.tile([2, F], BF16, tag="mfb")
                nc.gpsimd.dma_start(out=mfb, in_=fa[b])

                # combined mask first: PE work depends only on mtb/mfb,
                # so it overlaps the reduce chain below
                ms_halves = []
                for h in range(2):
                    msh = psp.tile([F, H], F32, tag="ms")
                    for c0 in range(0, H, 512):
                        cw = min(512, H - c0)
                        nc.tensor.matmul(
                            msh[:, c0 : c0 + cw],
                            mfb,
                            mtb[:, h * H + c0 : h * H + c0 + cw],
                            start=True,
                            stop=True,
                        )
                    ms_halves.append(msh)

                # per-sample min: free-axis reduce, gather across partitions
                colmin = sp.tile([F, 1], F32, tag="colmin")
                nc.vector.tensor_reduce(
                    out=colmin, in_=xt, axis=mybir.AxisListType.X,
                    op=mybir.AluOpType.min,
                )
                rowmin = sp.tile([1, F], F32, tag="rowmin")
                nc.gpsimd.dma_start(out=rowmin, in_=colmin)
                fill11 = sp.tile([1, 1], F32, tag="fill11")
                nc.vector.tensor_reduce(
                    out=fill11, in_=rowmin, axis=mybir.AxisListType.X,
                    op=mybir.AluOpType.min,
                )
                # fill broadcast [1,1] -> [1,128] (free) -> [128,1] (PE)
                fill_row = sp.tile([1, F], F32, tag="fill_row")
                nc.scalar.mul(fill_row, ones_row, fill11)
                fill128_ps = psps.tile([F, 1], F32, tag="fill128_ps")
                nc.tensor.matmul(fill128_ps, fill_row, one11, start=True, stop=True)
                fill128 = sp.tile([F, 1], F32, tag="fill128")
                nc.scalar.copy(fill128, fill128_ps)

                # nonzero mask => masked cell; overwrite with fill
                for h in range(2):
                    nc.vector.copy_predicated(
                        out=xt[:, h * H : (h + 1) * H],
                        mask=ms_halves[h].bitcast(mybir.dt.int32),
                        data=fill128.to_broadcast([F, H]),
                    )

                nc.scalar.dma_start(out=ya[b], in_=xt)
    nc.compile()
    return nc


def _host_masks(f0, f_w, t0, t_w):
    nb = f0.shape[0]
    fidx = np.arange(F, dtype=np.int32)
    tidx = np.arange(T, dtype=np.int32)
    fm = (
        (fidx[None, None, :] >= f0[:, :, None])
        & (fidx[None, None, :] < (f0 + f_w)[:, :, None])
    ).any(axis=1)  # [B,F] bool
    tm = (
        (tidx[None, None, :] >= t0[:, :, None])
        & (tidx[None, None, :] < (t0 + t_w)[:, :, None])
    ).any(axis=1)  # [B,T] bool
    mtr = np.ones((nb, 2, T), np.float32)
    mtr[:, 0, :] = tm
    mfl = np.ones((nb, 2, F), np.float32)
    mfl[:, 1, :] = fm
    return mtr.astype(ml_dtypes.bfloat16), mfl.astype(ml_dtypes.bfloat16)


def kernel(x, f0, f_w, t0, t_w, **_):
    x = np.ascontiguousarray(np.asarray(x, dtype=np.float32))
    f0 = np.asarray(f0)
    f_w = np.asarray(f_w)
    t0 = np.asarray(t0)
    t_w = np.asarray(t_w)
    mtr, mfl = _host_masks(f0, f_w, t0, t_w)

    if "nc" not in _cached:
        _cached["nc"] = _build_nc()
    nc = _cached["nc"]

    in_maps = []
    for c in range(N_CORES):
        s = slice(c * BPC, (c + 1) * BPC)
        in_maps.append(
            {
                "x_sh": np.ascontiguousarray(x[s]),
                "mtr_sh": np.ascontiguousarray(mtr[s]),
                "mfl_sh": np.ascontiguousarray(mfl[s]),
            }
        )
    res = bass_utils.run_bass_kernel_spmd(
        nc, in_maps, core_ids=list(range(N_CORES))
    )
    out = np.concatenate([r["y_sh"] for r in res.results], axis=0)
    return out


# revision 13
# speedup vs baseline: 1.8056x; 1.1608x over previous
"""SpecAugment (log-mel masking) Trainium2 kernel.

Full inputs: x [64,128,3000] f32, f0/f_w/t0/t_w [64,2] i32.
out[b,f,t] = fill_b if (f in freq band) or (t in time band) else x[b,f,t],
fill_b = min over x[b].

Strategy: batch-shard B=64 across 8 cores (8 samples/core). The int mask
params are tiny host tensors, so the per-sample 0/1 mask vectors are
computed on host and shipped as bf16 data; the device does only the
memory-bound work. Per sample:
  - DMA x[b] [128,3000] -> SBUF
  - DVE reduce_min (free axis) -> [128,1]; tiny DMA gather -> [1,128];
    reduce_min -> fill [1,1]; broadcast to [128,1] via tiny PE matmul
  - combined mask = ones(x)mt + mf(x)ones as ONE K=2 bf16 matmul per
    512-col chunk into PSUM (values {0,1,2}; nonzero == masked)
  - DVE copy_predicated overwrites masked cells with fill (data operand
    is fill128 broadcast along the free axis)
  - DMA xt -> y[b]
HBM traffic is the minimum 2 x 12.3MB per core -> ~69us roofline.
"""

import ml_dtypes
import numpy as np

import concourse.bacc as bacc
import concourse.bass as bass
import concourse.mybir as mybir
import concourse.tile as tile
import concourse.bass_utils as bass_utils

B, F, T = 64, 128, 3000
N_CORES = 8
BPC = B // N_CORES  # samples per core
F32 = mybir.dt.float32
BF16 = mybir.dt.bfloat16

_cached = {}


def _build_nc():
    nc = bacc.Bacc("TRN2", target_bir_lowering=False, debug=False)
    x = nc.dram_tensor("x_sh", [BPC, F, T], F32, kind="ExternalInput")
    # row0 = time mask (0/1), row1 = ones
    mtr = nc.dram_tensor("mtr_sh", [BPC, 2, T], BF16, kind="ExternalInput")
    # row0 = ones, row1 = freq mask (0/1)
    mfl = nc.dram_tensor("mfl_sh", [BPC, 2, F], BF16, kind="ExternalInput")
    y = nc.dram_tensor("y_sh", [BPC, F, T], F32, kind="ExternalOutput")

    xa, ta, fa, ya = x.ap(), mtr.ap(), mfl.ap(), y.ap()

    H = T // 2

    with tile.TileContext(nc) as tc:
        with (
            tc.tile_pool(name="xp", bufs=6) as xp,
            tc.tile_pool(name="row", bufs=6) as rowp,
            tc.tile_pool(name="small", bufs=6) as sp,
            tc.tile_pool(name="single", bufs=1) as single,
            tc.tile_pool(name="ps", bufs=2, space="PSUM") as psp,
            tc.tile_pool(name="ps_small", bufs=2, space="PSUM") as psps,
        ):
            ones_row = single.tile([1, F], F32)
            nc.vector.memset(ones_row, 1.0)
            one11 = single.tile([1, 1], F32)
            nc.vector.memset(one11, 1.0)

            for b in range(BPC):
                xt = xp.tile([F, T], F32, tag="xt")
                nc.sync.dma_start(out=xt, in_=xa[b])
                mtb = rowp.tile([2, T], BF16, tag="mtb")
                nc.gpsimd.dma_start(out=mtb, in_=ta[b])
                mfb = sp.tile([2, F], BF16, tag="mfb")
                nc.gpsimd.dma_start(out=mfb, in_=fa[b])

                # combined mask first: PE work depends only on mtb/mfb,
                # so it overlaps the reduce chain below
                ms_halves = []
                for h in range(2):
                    msh = psp.tile([F, H], F32, tag="ms")
                    for c0 in range(0, H, 512):
                        cw = min(512, H - c0)
                        nc.tensor.matmul(
                            msh[:, c0 : c0 + cw],
                            mfb,
                            mtb[:, h * H + c0 : h * H + c0 + cw],
                            start=True,
                            stop=True,
                        )
                    ms_halves.append(msh)

                # per-sample min: free-axis reduce, gather across partitions
                colmin = sp.tile([F, 1], F32, tag="colmin")
                nc.vector.tensor_reduce(
                    out=colmin, in_=xt, axis=mybir.AxisListType.X,
                    op=mybir.AluOpType.min,
                )
                rowmin = sp.tile([1, F], F32, tag="rowmin")
                nc.gpsimd.dma_start(out=rowmin, in_=colmin)
                fill11 = sp.tile([1, 1], F32, tag="fill11")
                nc.vector.tensor_reduce(
                    out=fill11, in_=rowmin, axis=mybir.AxisListType.X,
                    op=mybir.AluOpType.min,
                )
                # fill broadcast [1,1] -> [1,128] (free) -> [128,1] (PE)
                fill_row = sp.tile([1, F], F32, tag="fill_row")
                nc.scalar.mul(fill_row, ones_row, fill11)
                fill128_ps = psps.tile([F, 1], F32, tag="fill128_ps")
                nc.tensor.matmul(fill128_ps, fill_row, one11, start=True, stop=True)
                fill128 = sp.tile([F, 1], F32, tag="fill128")
                nc.scalar.copy(fill128, fill128_ps)

                # nonzero mask => masked cell; overwrite with fill, then
                # store each half as soon as its pred completes
                for h in range(2):
                    nc.vector.copy_predicated(
                        out=xt[:, h * H : (h + 1) * H],
                        mask=ms_halves[h].bitcast(mybir.dt.int32),
                        data=fill128.to_broadcast([F, H]),
                    )
                    nc.scalar.dma_start(
                        out=ya[b][:, h * H : (h + 1) * H],
                        in_=xt[:, h * H : (h + 1) * H],
                    )
    nc.compile()
    return nc


def _host_masks(f0, f_w, t0, t_w):
    nb = f0.shape[0]
    fidx = np.arange(F, dtype=np.int32)
    tidx = np.arange(T, dtype=np.int32)
    fm = (
        (fidx[None, None, :] >= f0[:, :, None])
        & (fidx[None, None, :] < (f0 + f_w)[:, :, None])
    ).any(axis=1)  # [B,F] bool
    tm = (
        (tidx[None, None, :] >= t0[:, :, None])
        & (tidx[None, None, :] < (t0 + t_w)[:, :, None])
    ).any(axis=1)  # [B,T] bool
    mtr = np.ones((nb, 2, T), np.float32)
    mtr[:, 0, :] = tm
    mfl = np.ones((nb, 2, F), np.float32)
    mfl[:, 1, :] = fm
    return mtr.astype(ml_dtypes.bfloat16), mfl.astype(ml_dtypes.bfloat16)


def kernel(x, f0, f_w, t0, t_w, **_):
    x = np.ascontiguousarray(np.asarray(x, dtype=np.float32))
    f0 = np.asarray(f0)
    f_w = np.asarray(f_w)
    t0 = np.asarray(t0)
    t_w = np.asarray(t_w)
    mtr, mfl = _host_masks(f0, f_w, t0, t_w)

    if "nc" not in _cached:
        _cached["nc"] = _build_nc()
    nc = _cached["nc"]

    in_maps = []
    for c in range(N_CORES):
        s = slice(c * BPC, (c + 1) * BPC)
        in_maps.append(
            {
                "x_sh": np.ascontiguousarray(x[s]),
                "mtr_sh": np.ascontiguousarray(mtr[s]),
                "mfl_sh": np.ascontiguousarray(mfl[s]),
            }
        )
    res = bass_utils.run_bass_kernel_spmd(
        nc, in_maps, core_ids=list(range(N_CORES))
    )
    out = np.concatenate([r["y_sh"] for r in res.results], axis=0)
    return out
